# revision 3
# baseline (speedup 1.0000x reference)
"""Trainium2 Bass kernel for nn_Actor_87497073754359.

Math (per batch b of B=128, x[b] is [N=2048, D=128] f32):
  graph_emb = mean_n x[b];  first/curr = x[b, idx]
  q = Wq @ (W_lin @ concat(graph_emb, first, curr) + b_lin) + bq  -> [H=8, HD=16]
  scores[h, n] = q[h] . (x @ Wk.T)[n, h*16:+16] / 4 ; mask; softmax over n
  out[b] = mean_h softmax

Never materialize k = x@Wk.T. Fold q into Wk:
  t[b][c, h] = sum_j Wk[j, c] * headsel_h(j) * q[b, j] * 0.25
  scores[b][h, n] = sum_c t[b][c, h] * xT[b][c, n]
x streams once as a host-pretransposed bf16 copy.

Layout: all 16 batches' heads share one PSUM tile per n-chunk of 512
(row = 8*b + h -> 128 rows).  Per chunk: one mask matmul (stationary
routes the 16 mask rows to all 128 psum rows), 16 per-batch score
matmuls (zero-padded [128,32] stationaries via PE column tiling), one
Exp, one combine matmul (rmat folds 1/Z and the 1/H head-average).
Row sums for the mean run on DVE (tensor_reduce), fully overlapped
with the x DMA stream; the PE only does scores + tiny q-chain, so it
tracks the stream.  1/N is folded into the host-combined Wq@W_lin.

Sharding: pure data parallel over batch (16/core), no collectives.
"""

import numpy as np
import ml_dtypes

import concourse.bass as bass
import concourse.tile as tile
from concourse import bacc, mybir
from concourse.bass_utils import run_bass_kernel_spmd
from concourse.masks import make_identity

B, N, D, H = 128, 2048, 128, 8
HD = D // H
NCORES = 8
BPC = B // NCORES          # 16 batches per core
P = 128
CH = 512                   # psum-bank chunk of n
NCH = N // CH              # 4
NQ = 4                     # batch quads per core
QS = BPC // NQ             # 4 batches per quad
MASKVAL = -1000.0          # exp(-1000 + s) == 0.0 exactly in f32

BF16 = mybir.dt.bfloat16
F32 = mybir.dt.float32
I32 = mybir.dt.int32


def build_kernel_body(ctx, tc):
    nc = tc.nc

    # ---- DRAM parameters (per-core shapes) ----
    xt = nc.dram_tensor("xt", [BPC, P, N], BF16, kind="ExternalInput")
    xn = nc.dram_tensor("xn", [BPC * N, D], BF16, kind="ExternalInput")
    gidx = nc.dram_tensor("gidx", [2 * BPC, 1], I32, kind="ExternalInput")
    maskneg = nc.dram_tensor("maskneg", [BPC, N], BF16, kind="ExternalInput")
    indmask = nc.dram_tensor("indmask", [P, P], BF16, kind="ExternalInput")
    ind16 = nc.dram_tensor("ind16", [P, BPC], BF16, kind="ExternalInput")
    wcombt = nc.dram_tensor("wcombt", [3, P, D], BF16, kind="ExternalInput")
    wk = nc.dram_tensor("wk", [D, D], BF16, kind="ExternalInput")
    headscat = nc.dram_tensor("headscat", [D, P], BF16, kind="ExternalInput")
    biasq = nc.dram_tensor("biasq", [D, 1], F32, kind="ExternalInput")
    out = nc.dram_tensor("out", [BPC, N], F32, kind="ExternalOutput")

    consts = ctx.enter_context(tc.tile_pool(name="consts", bufs=1))
    xt_pool = ctx.enter_context(tc.tile_pool(name="xt", bufs=BPC))
    small = ctx.enter_context(tc.tile_pool(name="small", bufs=2))
    w_pool = ctx.enter_context(tc.tile_pool(name="w", bufs=NCH))
    psum_small = ctx.enter_context(tc.tile_pool(name="ps_small", bufs=2, space="PSUM"))
    psum_scores = ctx.enter_context(
        tc.tile_pool(name="ps_scores", bufs=NCH, space="PSUM")
    )
    psum_out = ctx.enter_context(tc.tile_pool(name="ps_out", bufs=2, space="PSUM"))

    # ---- PE warm-up: ~4us of dense matmuls so HAM reaches 8/8 during DMA ----
    warm_src = consts.tile([P, CH], BF16)
    nc.vector.memset(warm_src, 1.0)
    for i in range(6):
        pw = psum_small.tile([P, CH], F32, tag="ps", name=f"warm{i}")
        nc.tensor.matmul(
            out=pw[:], lhsT=warm_src[:, :P], rhs=warm_src[:], start=True, stop=True
        )

    # ---- x stream first on the sync queue ----
    xt_tiles = []
    for b in range(BPC):
        xtb_t = xt_pool.tile([P, N], BF16, tag="xt", name=f"xt{b}")
        nc.sync.dma_start(xtb_t, xt[b])
        xt_tiles.append(xtb_t)

    # ---- constants into SBUF (scalar queue) ----
    maskneg_sb = consts.tile([P, N], BF16)
    nc.vector.memset(maskneg_sb, 0.0)
    nc.scalar.dma_start(maskneg_sb[:BPC, :], maskneg[:])
    indmask_sb = consts.tile([P, P], BF16)
    nc.scalar.dma_start(indmask_sb, indmask[:])
    wcombt_sb = consts.tile([P, 3, D], BF16)
    nc.scalar.dma_start(wcombt_sb, wcombt[:].rearrange("p c j -> c p j"))
    wk_sb = consts.tile([D, D], BF16)
    nc.scalar.dma_start(wk_sb, wk[:])
    headscat_sb = consts.tile([D, NQ, 32], BF16)
    nc.scalar.dma_start(headscat_sb[:].rearrange("d q x -> d (q x)"), headscat[:])
    biasq_sb = consts.tile([D, 1], F32)
    nc.scalar.dma_start(biasq_sb, biasq[:])
    ind16_sb = consts.tile([P, BPC], BF16)
    nc.scalar.dma_start(ind16_sb, ind16[:])
    gidx_sb = consts.tile([2 * BPC, 1], I32)
    nc.scalar.dma_start(gidx_sb, gidx[:])

    ident32 = consts.tile([2 * BPC, 2 * BPC], BF16)
    make_identity(nc, ident32[:])

    # ---- gather first/current node rows: [32, 128] -> featsT [128, 32] bf16 ----
    grows = consts.tile([2 * BPC, D], BF16)
    nc.gpsimd.indirect_dma_start(
        out=grows[:],
        out_offset=None,
        in_=xn[:],
        in_offset=bass.IndirectOffsetOnAxis(ap=gidx_sb[:, :1], axis=0),
    )
    psum_f = psum_small.tile([P, 2 * BPC], BF16, space="PSUM", tag="ps")
    nc.tensor.transpose(psum_f[:], grows[:], ident32[:])
    featsT_sb = consts.tile([P, 2 * BPC], BF16)
    nc.vector.tensor_copy(featsT_sb[:], psum_f[:])

    # ---- the 4 score psum tiles (one per n-chunk), mask matmul first ----
    score_ps = []
    for ch in range(NCH):
        ps = psum_scores.tile([P, CH], F32, space="PSUM", tag="pscore", name=f"sc{ch}")
        nc.tensor.matmul(
            out=ps[:],
            lhsT=indmask_sb[:],
            rhs=maskneg_sb[:, ch * CH : (ch + 1) * CH],
            start=True,
            stop=False,
            skip_group_check=True,
        )
        score_ps.append(ps)

    # ---- per quad: means (DVE), q-chain, then per batch 4 score matmuls ----
    sums_f32 = consts.tile([P, BPC], F32)
    sums_bf = consts.tile([P, BPC], BF16)
    statq_tiles = []
    for q in range(NQ):
        b0 = q * QS
        for b in range(b0, b0 + QS):
            nc.vector.tensor_reduce(
                out=sums_f32[:, b : b + 1],
                in_=xt_tiles[b][:],
                axis=mybir.AxisListType.X,
                op=mybir.AluOpType.add,
            )
        nc.vector.tensor_copy(sums_bf[:, b0 : b0 + QS], sums_f32[:, b0 : b0 + QS])

        psum_q = psum_small.tile([P, QS], F32, space="PSUM", tag="ps")
        ctx_chunks = [
            sums_bf[:, b0 : b0 + QS],
            featsT_sb[:, b0 : b0 + QS],
            featsT_sb[:, BPC + b0 : BPC + b0 + QS],
        ]
        for pch in range(3):
            nc.tensor.matmul(
                out=psum_q[:],
                lhsT=wcombt_sb[:, pch, :],
                rhs=ctx_chunks[pch],
                start=(pch == 0),
                stop=(pch == 2),
            )
        qb = small.tile([P, QS], BF16, tag="qb")
        nc.vector.tensor_scalar(
            out=qb[:],
            in0=psum_q[:],
            scalar1=biasq_sb[:, 0:1],
            scalar2=None,
            op0=mybir.AluOpType.add,
        )
        # qm[j, 32s + x] = headscat[j, s, x] * qb[j, s]; nonzero only at x = 8s+h
        qm = small.tile([P, NQ, 32], BF16, tag="qm")
        nc.vector.tensor_tensor(
            out=qm[:],
            in0=headscat_sb[:],
            in1=qb[:, :, None].to_broadcast([P, NQ, 32]),
            op=mybir.AluOpType.mult,
        )
        psum_t = psum_small.tile([P, NQ * 32], F32, space="PSUM", tag="ps")
        nc.tensor.matmul(
            out=psum_t[:],
            lhsT=wk_sb[:],
            rhs=qm[:].rearrange("p q x -> p (q x)"),
            start=True,
            stop=True,
        )
        statq = consts.tile([P, NQ * 32], BF16, name=f"statq{q}")
        nc.vector.tensor_copy(statq[:], psum_t[:])
        statq_tiles.append(statq)

        for s in range(QS):
            b = b0 + s
            for ch in range(NCH):
                nc.tensor.matmul(
                    out=score_ps[ch][32 * q : 32 * q + 32, :],
                    lhsT=statq[:, 32 * s : 32 * s + 32],
                    rhs=xt_tiles[b][:, ch * CH : (ch + 1) * CH],
                    start=False,
                    stop=(b == BPC - 1),
                    skip_group_check=True,
                    tile_position=(0, 32 * q),
                )

    # ---- exp (ACT), Z (DVE), rmat, combine (PE), out DMA from PSUM ----
    zpart = consts.tile([P, NCH], F32)
    ztot = consts.tile([P, 1], F32)
    recip = consts.tile([P, 1], F32)
    rmat = consts.tile([P, BPC], BF16)
    w_tiles = []
    for ch in range(NCH):
        wt = w_pool.tile([P, CH], BF16, tag="w", name=f"w{ch}")
        nc.scalar.activation(
            out=wt[:],
            in_=score_ps[ch][:],
            func=mybir.ActivationFunctionType.Exp,
        )
        nc.vector.tensor_reduce(
            out=zpart[:, ch : ch + 1],
            in_=wt[:],
            axis=mybir.AxisListType.X,
            op=mybir.AluOpType.add,
        )
        w_tiles.append(wt)
    nc.vector.tensor_reduce(
        out=ztot[:], in_=zpart[:], axis=mybir.AxisListType.X, op=mybir.AluOpType.add
    )
    nc.vector.reciprocal(recip[:], ztot[:])
    nc.vector.tensor_scalar(
        out=rmat[:],
        in0=ind16_sb[:],
        scalar1=recip[:, 0:1],
        scalar2=None,
        op0=mybir.AluOpType.mult,
    )
    out_sb = consts.tile([BPC, N], F32)
    for ch in range(NCH):
        psum_o = psum_out.tile([BPC, CH], F32, space="PSUM", tag="po")
        nc.tensor.matmul(
            out=psum_o[:], lhsT=rmat[:], rhs=w_tiles[ch][:], start=True, stop=True
        )
        nc.scalar.copy(out_sb[:, ch * CH : (ch + 1) * CH], psum_o[:])
        nc.sync.dma_start(
            out[:, ch * CH : (ch + 1) * CH], out_sb[:, ch * CH : (ch + 1) * CH]
        )


_NC_CACHE = None


def build_nc():
    global _NC_CACHE
    if _NC_CACHE is not None:
        return _NC_CACHE
    from contextlib import ExitStack

    nc = bacc.Bacc("TRN2", target_bir_lowering=False, debug=False)
    with tile.TileContext(nc) as tc:
        with ExitStack() as ctx:
            build_kernel_body(ctx, tc)
    nc.compile()
    _NC_CACHE = nc
    return nc


def make_in_maps(x, first_node, current_node, mask, W_lin, b_lin, Wq, bq, Wk, bk):
    """Host-side sharding/layout prep. Returns list of 8 per-core input dicts."""
    x = np.asarray(x, dtype=np.float32)
    mask = np.asarray(mask)
    first_node = np.asarray(first_node).astype(np.int32)
    current_node = np.asarray(current_node).astype(np.int32)
    W_lin = np.asarray(W_lin, dtype=np.float32)
    b_lin = np.asarray(b_lin, dtype=np.float32)
    Wq = np.asarray(Wq, dtype=np.float32)
    bq_v = np.asarray(bq, dtype=np.float32)
    Wk = np.asarray(Wk, dtype=np.float32)

    xbf = x.astype(ml_dtypes.bfloat16)

    # replicated weights; 1/N for the mean is folded into Wcomb chunk 0
    wcomb = (Wq @ W_lin).astype(np.float32)            # [D, 3D]
    wcomb[:, :D] *= 1.0 / N
    wcombt = np.ascontiguousarray(wcomb.T.reshape(3, P, D)).astype(ml_dtypes.bfloat16)
    biasq = (Wq @ b_lin + bq_v).astype(np.float32).reshape(D, 1)
    wk_in = np.ascontiguousarray(Wk).astype(ml_dtypes.bfloat16)

    # headscat[j, 32s + 8s + h] = head-h indicator * 1/sqrt(HD); zeros elsewhere.
    # Column block s (32 wide) is the zero-padded stationary slot for the quad's
    # batch s; within it the batch's 8 head-columns sit at offset 8s.
    headscat = np.zeros((D, P), dtype=np.float32)
    for s in range(QS):
        for h in range(H):
            for j in range(D):
                if j // HD == h:
                    headscat[j, 32 * s + 8 * s + h] = 1.0 / np.sqrt(HD)
    headscat = headscat.astype(ml_dtypes.bfloat16)

    # indmask[r, 8b + h] = 1 if r == b: routes mask row b to its 8 psum rows
    indmask = np.zeros((P, P), dtype=np.float32)
    for b in range(BPC):
        for h in range(H):
            indmask[b, 8 * b + h] = 1.0
    indmask = indmask.astype(ml_dtypes.bfloat16)

    # ind16[8b + h, b] = 1/H: combine folds the head average (1/Z via recip)
    ind16 = np.zeros((P, BPC), dtype=np.float32)
    for b in range(BPC):
        for h in range(H):
            ind16[8 * b + h, b] = 1.0 / H
    ind16 = ind16.astype(ml_dtypes.bfloat16)

    in_maps = []
    for c in range(NCORES):
        lo = c * BPC
        xs = xbf[lo : lo + BPC]                               # [16, 2048, 128]
        xtc = np.ascontiguousarray(xs.transpose(0, 2, 1))     # [16, 128, 2048]
        xnc = np.ascontiguousarray(xs.reshape(BPC * N, D))
        gi = np.concatenate(
            [
                np.arange(BPC, dtype=np.int32) * N + first_node[lo : lo + BPC, 0],
                np.arange(BPC, dtype=np.int32) * N + current_node[lo : lo + BPC, 0],
            ]
        ).reshape(2 * BPC, 1).astype(np.int32)
        mneg = (mask[lo : lo + BPC].astype(np.float32) * MASKVAL).astype(
            ml_dtypes.bfloat16
        )
        in_maps.append(
            {
                "xt": xtc,
                "xn": xnc,
                "gidx": gi,
                "maskneg": mneg,
                "indmask": indmask,
                "ind16": ind16,
                "wcombt": wcombt,
                "wk": wk_in,
                "headscat": headscat,
                "biasq": biasq,
            }
        )
    return in_maps


def kernel(**inputs) -> np.ndarray:
    nc = build_nc()
    in_maps = make_in_maps(**inputs)
    res = run_bass_kernel_spmd(nc, in_maps, core_ids=list(range(NCORES)))
    outs = [np.asarray(res.results[c]["out"]) for c in range(NCORES)]
    return np.concatenate(outs, axis=0)


# revision 10
# speedup vs baseline: 1.1093x; 1.1093x over previous
"""Trainium2 Bass kernel for nn_Actor_87497073754359.

Math (per batch b of B=128, x[b] is [N=2048, D=128] f32):
  graph_emb = mean_n x[b];  first/curr = x[b, idx]
  q = Wq @ (W_lin @ concat(graph_emb, first, curr) + b_lin) + bq  -> [H=8, HD=16]
  scores[h, n] = q[h] . (x @ Wk.T)[n, h*16:+16] / 4 ; mask; softmax over n
  out[b] = mean_h softmax

Never materialize k = x@Wk.T. Fold q into Wk:
  t[b][c, h] = sum_j Wk[j, c] * headsel_h(j) * q[b, j] * 0.25
  scores[b][h, n] = sum_c t[b][c, h] * xT[b][c, n]
x streams once as a host-pretransposed bf16 copy, split across the
sync (even tiles) and gpsimd (odd tiles) queues.

Layout: all 16 batches' heads share one PSUM tile per n-chunk of 512
(row = 8*b + h -> 128 rows).  Per chunk: one mask matmul (stationary
routes the 16 mask rows to all 128 psum rows), 16 per-batch score
matmuls (zero-padded [128,32] stationaries via PE column tiling), one
Exp, one combine matmul (rmat folds 1/Z and the 1/H head-average).
Row sums for the mean are spread over DVE/ACT/GPSIMD (one each + one
ACT per quad), fully overlapped with the x DMA stream; the last batch
streams as two half-tiles so its mean partials pipeline with the DMA.
1/N is folded into the host-combined Wq@W_lin weight.

Sharding: pure data parallel over batch (16/core), no collectives.
"""

import numpy as np
import ml_dtypes

import concourse.bass as bass
import concourse.tile as tile
from concourse import bacc, mybir
from concourse.bass_utils import run_bass_kernel_spmd
from concourse.masks import make_identity

B, N, D, H = 128, 2048, 128, 8
HD = D // H
NCORES = 8
BPC = B // NCORES          # 16 batches per core
P = 128
CH = 512                   # psum-bank chunk of n
NCH = N // CH              # 4
NQ = 4                     # batch quads per core
QS = BPC // NQ             # 4 batches per quad
LASTB = BPC - 1
MASKVAL = -1000.0          # exp(-1000 + s) == 0.0 exactly in f32

BF16 = mybir.dt.bfloat16
F32 = mybir.dt.float32
I32 = mybir.dt.int32


def build_kernel_body(ctx, tc):
    nc = tc.nc

    # ---- DRAM parameters (per-core shapes) ----
    xt = nc.dram_tensor("xt", [BPC, P, N], BF16, kind="ExternalInput")
    xn = nc.dram_tensor("xn", [BPC * N, D], BF16, kind="ExternalInput")
    gidx = nc.dram_tensor("gidx", [2 * BPC, 1], I32, kind="ExternalInput")
    maskneg = nc.dram_tensor("maskneg", [BPC, N], BF16, kind="ExternalInput")
    indmask = nc.dram_tensor("indmask", [P, P], BF16, kind="ExternalInput")
    ind16 = nc.dram_tensor("ind16", [P, BPC], BF16, kind="ExternalInput")
    wcombt = nc.dram_tensor("wcombt", [3, P, D], BF16, kind="ExternalInput")
    wk = nc.dram_tensor("wk", [D, D], BF16, kind="ExternalInput")
    headscat = nc.dram_tensor("headscat", [D, P], BF16, kind="ExternalInput")
    biasq = nc.dram_tensor("biasq", [D, 1], F32, kind="ExternalInput")
    out = nc.dram_tensor("out", [BPC, N], F32, kind="ExternalOutput")

    consts = ctx.enter_context(tc.tile_pool(name="consts", bufs=1))
    xt_pool = ctx.enter_context(tc.tile_pool(name="xt", bufs=BPC))
    small = ctx.enter_context(tc.tile_pool(name="small", bufs=2))
    mscr_pool = ctx.enter_context(tc.tile_pool(name="mscr", bufs=2))
    w_pool = ctx.enter_context(tc.tile_pool(name="w", bufs=NCH))
    psum_small = ctx.enter_context(tc.tile_pool(name="ps_small", bufs=2, space="PSUM"))
    psum_scores = ctx.enter_context(
        tc.tile_pool(name="ps_scores", bufs=NCH, space="PSUM")
    )
    psum_out = ctx.enter_context(tc.tile_pool(name="ps_out", bufs=2, space="PSUM"))

    # ---- gather chain first on the gpsimd queue (it gates every q-chain) ----
    gidx_sb = consts.tile([2 * BPC, 1], I32)
    nc.gpsimd.dma_start(gidx_sb, gidx[:])
    grows = consts.tile([2 * BPC, D], BF16)
    nc.gpsimd.indirect_dma_start(
        out=grows[:],
        out_offset=None,
        in_=xn[:],
        in_offset=bass.IndirectOffsetOnAxis(ap=gidx_sb[:, :1], axis=0),
    )

    # ---- x stream: even tiles on sync queue, odd tiles on gpsimd queue ----
    # The last batch arrives as two half-tiles so its row-sum partials can
    # pipeline with the stream (it is on everyone's critical path).
    xt_tiles = []
    for b in range(BPC):
        xtb_t = xt_pool.tile([P, N], BF16, tag="xt", name=f"xt{b}")
        xt_tiles.append(xtb_t)
    for b in range(0, BPC, 2):
        if b == LASTB:
            continue
        nc.sync.dma_start(xt_tiles[b], xt[b])
    for b in range(1, BPC, 2):
        if b == LASTB:
            continue
        nc.gpsimd.dma_start(xt_tiles[b], xt[b])
    nc.sync.dma_start(xt_tiles[LASTB][:, : N // 2], xt[LASTB, :, : N // 2])
    nc.sync.dma_start(xt_tiles[LASTB][:, N // 2 :], xt[LASTB, :, N // 2 :])

    # ---- constants into SBUF (scalar queue), in dependency-priority order ----
    maskneg_sb = consts.tile([P, N], BF16)
    nc.vector.memset(maskneg_sb, 0.0)
    nc.scalar.dma_start(maskneg_sb[:BPC, :], maskneg[:])
    indmask_sb = consts.tile([P, P], BF16)
    nc.scalar.dma_start(indmask_sb, indmask[:])
    wcombt_sb = consts.tile([P, 3, D], BF16)
    nc.scalar.dma_start(wcombt_sb, wcombt[:].rearrange("p c j -> c p j"))
    wk_sb = consts.tile([D, D], BF16)
    nc.scalar.dma_start(wk_sb, wk[:])
    headscat_sb = consts.tile([D, NQ, 32], BF16)
    nc.scalar.dma_start(headscat_sb[:].rearrange("d q x -> d (q x)"), headscat[:])
    biasq_sb = consts.tile([D, 1], F32)
    nc.scalar.dma_start(biasq_sb, biasq[:])
    ind16_sb = consts.tile([P, BPC], BF16)
    nc.scalar.dma_start(ind16_sb, ind16[:])

    # ---- PE warm-up: ~4us of dense matmuls so HAM reaches 8/8 early ----
    warm_src = consts.tile([P, CH], BF16)
    nc.vector.memset(warm_src, 1.0)
    for i in range(6):
        pw = psum_small.tile([P, CH], F32, tag="ps", name=f"warm{i}")
        nc.tensor.matmul(
            out=pw[:], lhsT=warm_src[:, :P], rhs=warm_src[:], start=True, stop=True
        )

    # ---- the 4 score psum tiles (one per n-chunk), mask matmul first ----
    score_ps = []
    for ch in range(NCH):
        ps = psum_scores.tile([P, CH], F32, space="PSUM", tag="pscore", name=f"sc{ch}")
        nc.tensor.matmul(
            out=ps[:],
            lhsT=indmask_sb[:],
            rhs=maskneg_sb[:, ch * CH : (ch + 1) * CH],
            start=True,
            stop=False,
            skip_group_check=True,
        )
        score_ps.append(ps)

    # ---- gathered rows -> featsT [128, 32] bf16 (transpose on PE) ----
    ident32 = consts.tile([2 * BPC, 2 * BPC], BF16)
    make_identity(nc, ident32[:])
    ident128 = consts.tile([P, P], BF16)
    make_identity(nc, ident128[:])
    psum_f = psum_small.tile([P, 2 * BPC], BF16, space="PSUM", tag="ps")
    nc.tensor.transpose(psum_f[:], grows[:], ident32[:])
    featsT_sb = consts.tile([P, 2 * BPC], BF16)
    nc.vector.tensor_copy(featsT_sb[:], psum_f[:])

    # ---- per quad: means (DVE/ACT/GPSIMD), q-chain, per-batch score MMs ----
    # sums_f32 col b = row-sum of batch b; col BPC = second partial of LASTB
    sums_f32 = consts.tile([P, BPC + 1], F32)
    sums_bf = consts.tile([P, BPC + 1], BF16)

    def emit_mean_dve(b, lo, hi, col):
        nc.vector.tensor_reduce(
            out=sums_f32[:, col : col + 1],
            in_=xt_tiles[b][:, lo:hi],
            axis=mybir.AxisListType.X,
            op=mybir.AluOpType.add,
        )

    def emit_mean_act(b, lo, hi, col):
        scr = mscr_pool.tile([P, N], BF16, tag="mscr")
        nc.scalar.activation(
            out=scr[:, lo:hi],
            in_=xt_tiles[b][:, lo:hi],
            func=mybir.ActivationFunctionType.Copy,
            accum_out=sums_f32[:, col : col + 1],
        )

    def emit_mean_pe(b, col):
        # identity-stationary matmul: psum[:, j] accumulates x[:, k*512 + j]
        pm = psum_small.tile([P, CH], F32, space="PSUM", tag="ps", name=f"pm{b}")
        for k in range(NCH):
            nc.tensor.matmul(
                out=pm[:],
                lhsT=ident128[:],
                rhs=xt_tiles[b][:, k * CH : (k + 1) * CH],
                start=(k == 0),
                stop=(k == NCH - 1),
            )
        pescr = mscr_pool.tile([P, CH], BF16, tag="pescr", name=f"pescr{b}")
        nc.scalar.activation(
            out=pescr[:],
            in_=pm[:],
            func=mybir.ActivationFunctionType.Copy,
            accum_out=sums_f32[:, col : col + 1],
        )

    statq_tiles = []
    for q in range(NQ):
        b0 = q * QS
        # means: batch b0 -> DVE, b0+1/b0+2 -> ACT, b0+3 -> GPSIMD
        # (the very last batch: two ACT halves, pipelined with its DMA)
        emit_mean_act(b0 + 1, 0, N, b0 + 1)
        emit_mean_act(b0 + 2, 0, N, b0 + 2)
        emit_mean_dve(b0, 0, N, b0)
        if b0 + 3 == LASTB:
            emit_mean_act(LASTB, 0, N // 2, LASTB)
            emit_mean_act(LASTB, N // 2, N, BPC)
        else:
            emit_mean_pe(b0 + 3, b0 + 3)

        nc.vector.tensor_copy(
            sums_bf[:, b0 : b0 + QS + (1 if b0 + 3 == LASTB else 0)],
            sums_f32[:, b0 : b0 + QS + (1 if b0 + 3 == LASTB else 0)],
        )

        psum_q = psum_small.tile([P, QS], F32, space="PSUM", tag="ps")
        ctx_chunks = [
            sums_bf[:, b0 : b0 + QS],
            featsT_sb[:, b0 : b0 + QS],
            featsT_sb[:, BPC + b0 : BPC + b0 + QS],
        ]
        has_extra = b0 + 3 == LASTB
        for pch in range(3):
            nc.tensor.matmul(
                out=psum_q[:],
                lhsT=wcombt_sb[:, pch, :],
                rhs=ctx_chunks[pch],
                start=(pch == 0),
                stop=(pch == 2 and not has_extra),
                skip_group_check=True,
            )
        if has_extra:
            # second half-sum of the last batch folds in via one FD=1 matmul
            nc.tensor.matmul(
                out=psum_q[:, QS - 1 : QS],
                lhsT=wcombt_sb[:, 0, :],
                rhs=sums_bf[:, BPC : BPC + 1],
                start=False,
                stop=True,
                skip_group_check=True,
            )
        qb = small.tile([P, QS], BF16, tag="qb")
        nc.vector.tensor_scalar(
            out=qb[:],
            in0=psum_q[:],
            scalar1=biasq_sb[:, 0:1],
            scalar2=None,
            op0=mybir.AluOpType.add,
        )
        # qm[j, 32s + x] = headscat[j, s, x] * qb[j, s]; nonzero only at x = 8s+h
        qm = small.tile([P, NQ, 32], BF16, tag="qm")
        nc.vector.tensor_tensor(
            out=qm[:],
            in0=headscat_sb[:],
            in1=qb[:, :, None].to_broadcast([P, NQ, 32]),
            op=mybir.AluOpType.mult,
        )
        psum_t = psum_small.tile([P, NQ * 32], F32, space="PSUM", tag="ps")
        nc.tensor.matmul(
            out=psum_t[:],
            lhsT=wk_sb[:],
            rhs=qm[:].rearrange("p q x -> p (q x)"),
            start=True,
            stop=True,
        )
        statq = consts.tile([P, NQ * 32], BF16, name=f"statq{q}")
        nc.vector.tensor_copy(statq[:], psum_t[:])
        statq_tiles.append(statq)

        for s in range(QS):
            b = b0 + s
            for ch in range(NCH):
                nc.tensor.matmul(
                    out=score_ps[ch][32 * q : 32 * q + 32, :],
                    lhsT=statq[:, 32 * s : 32 * s + 32],
                    rhs=xt_tiles[b][:, ch * CH : (ch + 1) * CH],
                    start=False,
                    stop=(b == BPC - 1),
                    skip_group_check=True,
                    tile_position=(0, 32 * q),
                )

    # ---- exp (ACT), Z (DVE), rmat, combine (PE), copy out, DMA ----
    zpart = consts.tile([P, NCH], F32)
    ztot = consts.tile([P, 1], F32)
    recip = consts.tile([P, 1], F32)
    rmat = consts.tile([P, BPC], BF16)
    w_tiles = []
    for ch in range(NCH):
        wt = w_pool.tile([P, CH], BF16, tag="w", name=f"w{ch}")
        nc.scalar.activation(
            out=wt[:],
            in_=score_ps[ch][:],
            func=mybir.ActivationFunctionType.Exp,
        )
        nc.vector.tensor_reduce(
            out=zpart[:, ch : ch + 1],
            in_=wt[:],
            axis=mybir.AxisListType.X,
            op=mybir.AluOpType.add,
        )
        w_tiles.append(wt)
    nc.vector.tensor_reduce(
        out=ztot[:], in_=zpart[:], axis=mybir.AxisListType.X, op=mybir.AluOpType.add
    )
    nc.vector.reciprocal(recip[:], ztot[:])
    nc.vector.tensor_scalar(
        out=rmat[:],
        in0=ind16_sb[:],
        scalar1=recip[:, 0:1],
        scalar2=None,
        op0=mybir.AluOpType.mult,
    )
    out_sb = consts.tile([BPC, N], F32)
    for ch in range(NCH):
        psum_o = psum_out.tile([BPC, CH], F32, space="PSUM", tag="po")
        nc.tensor.matmul(
            out=psum_o[:], lhsT=rmat[:], rhs=w_tiles[ch][:], start=True, stop=True
        )
        cp = nc.scalar.copy if ch % 2 == 0 else nc.vector.tensor_copy
        cp(out_sb[:, ch * CH : (ch + 1) * CH], psum_o[:])
        nc.sync.dma_start(
            out[:, ch * CH : (ch + 1) * CH], out_sb[:, ch * CH : (ch + 1) * CH]
        )


_NC_CACHE = None


def build_nc():
    global _NC_CACHE
    if _NC_CACHE is not None:
        return _NC_CACHE
    from contextlib import ExitStack

    nc = bacc.Bacc("TRN2", target_bir_lowering=False, debug=False)
    with tile.TileContext(nc) as tc:
        with ExitStack() as ctx:
            build_kernel_body(ctx, tc)
    nc.compile()
    _NC_CACHE = nc
    return nc


def make_in_maps(x, first_node, current_node, mask, W_lin, b_lin, Wq, bq, Wk, bk):
    """Host-side sharding/layout prep. Returns list of 8 per-core input dicts."""
    x = np.asarray(x, dtype=np.float32)
    mask = np.asarray(mask)
    first_node = np.asarray(first_node).astype(np.int32)
    current_node = np.asarray(current_node).astype(np.int32)
    W_lin = np.asarray(W_lin, dtype=np.float32)
    b_lin = np.asarray(b_lin, dtype=np.float32)
    Wq = np.asarray(Wq, dtype=np.float32)
    bq_v = np.asarray(bq, dtype=np.float32)
    Wk = np.asarray(Wk, dtype=np.float32)

    xbf = x.astype(ml_dtypes.bfloat16)

    # replicated weights; 1/N for the mean is folded into Wcomb chunk 0
    wcomb = (Wq @ W_lin).astype(np.float32)            # [D, 3D]
    wcomb[:, :D] *= 1.0 / N
    wcombt = np.ascontiguousarray(wcomb.T.reshape(3, P, D)).astype(ml_dtypes.bfloat16)
    biasq = (Wq @ b_lin + bq_v).astype(np.float32).reshape(D, 1)
    wk_in = np.ascontiguousarray(Wk).astype(ml_dtypes.bfloat16)

    # headscat[j, 32s + 8s + h] = head-h indicator * 1/sqrt(HD); zeros elsewhere.
    # Column block s (32 wide) is the zero-padded stationary slot for the quad's
    # batch s; within it the batch's 8 head-columns sit at offset 8s.
    headscat = np.zeros((D, P), dtype=np.float32)
    for s in range(QS):
        for h in range(H):
            for j in range(D):
                if j // HD == h:
                    headscat[j, 32 * s + 8 * s + h] = 1.0 / np.sqrt(HD)
    headscat = headscat.astype(ml_dtypes.bfloat16)

    # indmask[r, 8b + h] = 1 if r == b: routes mask row b to its 8 psum rows
    indmask = np.zeros((P, P), dtype=np.float32)
    for b in range(BPC):
        for h in range(H):
            indmask[b, 8 * b + h] = 1.0
    indmask = indmask.astype(ml_dtypes.bfloat16)

    # ind16[8b + h, b] = 1/H: combine folds the head average (1/Z via recip)
    ind16 = np.zeros((P, BPC), dtype=np.float32)
    for b in range(BPC):
        for h in range(H):
            ind16[8 * b + h, b] = 1.0 / H
    ind16 = ind16.astype(ml_dtypes.bfloat16)

    in_maps = []
    for c in range(NCORES):
        lo = c * BPC
        xs = xbf[lo : lo + BPC]                               # [16, 2048, 128]
        xtc = np.ascontiguousarray(xs.transpose(0, 2, 1))     # [16, 128, 2048]
        xnc = np.ascontiguousarray(xs.reshape(BPC * N, D))
        gi = np.concatenate(
            [
                np.arange(BPC, dtype=np.int32) * N + first_node[lo : lo + BPC, 0],
                np.arange(BPC, dtype=np.int32) * N + current_node[lo : lo + BPC, 0],
            ]
        ).reshape(2 * BPC, 1).astype(np.int32)
        mneg = (mask[lo : lo + BPC].astype(np.float32) * MASKVAL).astype(
            ml_dtypes.bfloat16
        )
        in_maps.append(
            {
                "xt": xtc,
                "xn": xnc,
                "gidx": gi,
                "maskneg": mneg,
                "indmask": indmask,
                "ind16": ind16,
                "wcombt": wcombt,
                "wk": wk_in,
                "headscat": headscat,
                "biasq": biasq,
            }
        )
    return in_maps


def kernel(**inputs) -> np.ndarray:
    nc = build_nc()
    in_maps = make_in_maps(**inputs)
    res = run_bass_kernel_spmd(nc, in_maps, core_ids=list(range(NCORES)))
    outs = [np.asarray(res.results[c]["out"]) for c in range(NCORES)]
    return np.concatenate(outs, axis=0)
